# revision 53
# baseline (speedup 1.0000x reference)
"""Distributed GQA attention block for Trainium2 (8 NeuronCores).

Problem: nn_Attention_65927747993826
  x:[2,2048,2048] f32, causal GQA attention, H=32 query heads, G=8 KV groups,
  head_size=64, with q/k/v/out projections and bias.

Sharding (8-way head parallel): core c owns query heads [4c, 4c+4) and KV
group c. Each core computes q/k/v projections for its heads from the full x,
causal flash-attention for its 4 heads, and a partial output projection
through its 256 rows of Wo. The host sums the 8 partial outputs and adds the
bias (a per-feature constant commutes with the partial-sum reduction).

Layouts on chip are feature-major ("transposed"): x^T [E, S] etc., so every
matmul contracts over the partition dim with zero on-chip transposes except
v (PE-transposed). Compute dtype bf16 (f32 accumulate in PSUM).
"""

from contextlib import ExitStack

import numpy as np
import ml_dtypes

import concourse.bass as bass
import concourse.mybir as mybir
import concourse.tile as tile
from concourse import bacc
from concourse.bass import ts, ds
from concourse.bass_utils import run_bass_kernel_spmd
from concourse.masks import make_identity, make_upper_triangular

B, S, E = 2, 2048, 2048
H, G, D = 32, 8, 64
NCORES = 8
HPC = H // NCORES            # query heads per core: 4
FPC = HPC * D                # q features per core: 256
P = 128
KT = E // P                  # 16 contraction tiles over E
NT = S // 512                # 4 token 512-blocks per batch
SCALE = D ** -0.5
F32 = mybir.dt.float32
BF16 = mybir.dt.bfloat16
FA = mybir.ActivationFunctionType
ALU = mybir.AluOpType


def build_nc():
    nc = bacc.Bacc()
    # x_t tiled [B, NT, E, 512]: each (b, n) token-block is a dense 2MB
    # region so the strided per-partition DMA rows stay page-local
    x_t = nc.declare_dram_parameter("x_t", [B, NT, E, 512], BF16, isOutput=False)
    wq = nc.declare_dram_parameter("wq", [E, FPC], BF16, isOutput=False)
    wkv = nc.declare_dram_parameter("wkv", [E, P], BF16, isOutput=False)
    wo = nc.declare_dram_parameter("wo", [FPC, E], BF16, isOutput=False)
    # out tiled [B, NT, 128, KT, 512]: feature-tile-minor so one DMA covers
    # TWO consecutive m-tiles as a partition-major [128, 1024] transfer —
    # half the ~590ns-descriptor cost per byte; the host reassembles
    out = nc.declare_dram_parameter("out", [B, NT, P, KT, 512], BF16, isOutput=True)

    with ExitStack() as ctx:
        tc = ctx.enter_context(tile.TileContext(nc))
        consts = ctx.enter_context(tc.tile_pool(name="consts", bufs=1))
        wpool = ctx.enter_context(tc.tile_pool(name="w", bufs=1))
        xbp = ctx.enter_context(tc.tile_pool(name="xb", bufs=6))
        qkvp = ctx.enter_context(tc.tile_pool(name="qkv", bufs=2))
        ppool = ctx.enter_context(tc.tile_pool(name="probs", bufs=18))
        npool = ctx.enter_context(tc.tile_pool(name="norm", bufs=2))
        opool = ctx.enter_context(tc.tile_pool(name="outsb", bufs=4))
        pp_mm = ctx.enter_context(tc.tile_pool(name="pmm", bufs=2, space="PSUM"))
        pp_sp = ctx.enter_context(tc.tile_pool(name="psp", bufs=2, space="PSUM"))
        pp_acc = ctx.enter_context(tc.tile_pool(name="pacc", bufs=2, space="PSUM"))

        # ---- weights (gpsimd DMA queue, parallel to the x loads on sync;
        # emitted BEFORE the constants so their descriptors aren't delayed
        # behind the gpsimd memset/iota chain) ----
        # wq in 4 ko-chunks so the opening q-proj matmuls only wait on the
        # first 256KB instead of the full megabyte
        wq_sb = wpool.tile([P, KT, FPC], BF16)
        wq_r = wq.rearrange("(ko p) m -> p ko m", p=P)
        for ch in range(4):
            nc.gpsimd.dma_start(wq_sb[:, ds(4 * ch, 4), :], wq_r[:, ds(4 * ch, 4), :])
        wkv_sb = wpool.tile([P, KT, P], BF16)
        nc.gpsimd.dma_start(wkv_sb, wkv.rearrange("(ko p) m -> p ko m", p=P))
        # wo isn't needed until the first out-projection (~60us in): emit its
        # DMA after the constants so its 1MB doesn't contend with the
        # x-stream during the DMA-bandwidth-limited opening ramp
        wo_sb = wpool.tile([P, 2, E], BF16)

        # ---- constants ----
        ident = consts.tile([P, P], BF16)
        make_identity(nc, ident)
        tri = consts.tile([P, P], BF16)  # tri[k, q] = 1 iff q >= k
        make_upper_triangular(nc, tri, val=1.0, diag=True)
        # sel2 row 0 selects partitions 0-63, row 32 selects 64-127 (rows are
        # 32-aligned — the engines reject other base partitions). lhsT of the
        # K=64 broadcast matmul that fans the per-head 1/denom rows out to the
        # 64-partition feature blocks of attnsb.
        sel2 = consts.tile([64, P], BF16)
        nc.gpsimd.memset(sel2, 0.0)
        nc.gpsimd.memset(sel2[0:1, 0:64], 1.0)
        nc.gpsimd.memset(sel2[32:33, 64:128], 1.0)

        nc.gpsimd.dma_start(wo_sb, wo.rearrange("(ko p) m -> p ko m", p=P))

        state = {
            ("normed", 0): [0], ("normed", 1): [0],
            ("projed", 0): [0], ("projed", 1): [0],
        }

        def gen_proj_n(b, n):
            """projection of token-block n for batch b (PE-heavy)."""
            if n == 0:
                # q4[:, p, :]: head-pair p with head 2p's q^T at partitions
                # 0-63 and head 2p+1's at 64-127 — the scores matmuls run
                # K=64 row-tiled, both heads concurrently in the two array
                # halves.
                q4 = qkvp.tile([P, 2, S], BF16, tag="q4")
                # kdup: k^T duplicated into both partition halves (row-tiled
                # scores lhsT for tiles (0,0) and (64,0))
                kdup = qkvp.tile([P, S], BF16, tag="kdup")
                # kv: k^T rows 0-63, v^T rows 64-127 (v-transpose source)
                kvsb = qkvp.tile([P, S], BF16, tag="kv")
                # v token-major (+ ones column), PE-transposed per block
                vsb = qkvp.tile([P, S // P, D + 1], BF16, tag="v")
                nc.gpsimd.memset(vsb[:, :, D : D + 1], 1.0)
                # attnsb holds UNNORMALIZED attn until deferred normalize
                attnsb = qkvp.tile([P, 2, S], BF16, tag="attn")
                state[b] = (q4, kdup, kvsb, vsb, attnsb)
            q4, kdup, kvsb, vsb, attnsb = state[b]
            # two half-tiles with separate DMAs: the first 8 k-tiles of
            # matmul only wait on the first half's DMA. The very first
            # block is DMA'd in quarters so the opening matmul can start
            # as early as possible.
            xh = []
            nchunk = 4 if (b == 0 and n == 0) else 1
            # spread the half-block DMAs across the two hwdge engine queues
            # (and quarter the very first block) so transfers run in
            # parallel instead of serializing on the sync queue
            qeng = [nc.sync, nc.scalar, nc.sync, nc.scalar]
            for g in range(2):
                xb = xbp.tile([P, KT // 2, 512], BF16)
                for q_ in range(nchunk):
                    eng = qeng[q_] if nchunk > 1 else (nc.sync, nc.scalar)[g]
                    eng.dma_start(
                        xb[:, ds(q_ * (8 // nchunk), 8 // nchunk), :],
                        x_t[
                            b, n,
                            ds(1024 * g + q_ * (1024 // nchunk), 1024 // nchunk),
                            :,
                        ].rearrange("(ko p) s -> p ko s", p=P),
                    )
                xh.append(xb)
            for m in range(3):
                ps = pp_mm.tile([P, 512], F32, tag="mm")
                for k in range(KT):
                    lhsT = wq_sb[:, k, ts(m, P)] if m < 2 else wkv_sb[:, k, :]
                    nc.tensor.matmul(
                        ps,
                        lhsT,
                        xh[k // 8][:, k % 8, :],
                        start=(k == 0),
                        stop=(k == KT - 1),
                    )
                if m < 2:
                    # pair layout matches the psum exactly: one wide copy
                    nc.vector.tensor_copy(q4[:, m, ts(n, 512)], ps)
                else:
                    nc.vector.tensor_copy(kvsb[:, ts(n, 512)], ps)
                    # duplicate k into both halves of kdup via DMA off the
                    # idle gpsimd queue (no DVE cost)
                    nc.gpsimd.dma_start(kdup[0:64, ts(n, 512)], kvsb[0:64, ts(n, 512)])
                    nc.gpsimd.dma_start(kdup[64:128, ts(n, 512)], kvsb[0:64, ts(n, 512)])
                yield
            # v transposes for this token-block, done per-block so a proj
            # generator that finishes block n has also produced vsb block n —
            # lets proj(1)'s later blocks run as phase-3 filler without the
            # attention racing ahead of its vsb inputs in the PE queue
            for st in range(4 * n, 4 * n + 4):
                tp = pp_mm.tile([P, P], BF16, tag="mm")
                nc.tensor.transpose(tp, kvsb[:, ts(st, P)], ident)
                nc.vector.tensor_copy(vsb[:, st, 0:D], tp[:, 64:128])
            state[("projed", b)][0] += 1
            yield

        def gen_att_qt(b, qt):
            """causal attention for q-block qt of batch b (ACT-heavy: exp).

            softmax denominators come for free from the ones-column of v_aug
            (row 64 of each accumulator); per-head approximate reciprocals
            run straight from PSUM, and the rest of the normalization (bf16
            cast + selector-matmul broadcast + multiplies) is DEFERRED into
            the next q-block's instruction stream (gen_norm), so the bc
            matmul never stalls the in-order PE queue waiting on the DVE
            reciprocal chain.
            """
            q4, kdup, kvsb, vsb, attnsb = state[b]
            if True:
                # dq[32*hh, 512*p + q] = denominator of head 2p+hh at query q
                # (rows 32-aligned). The 1.0 background keeps the reciprocal
                # finite on the unused rows so the selector matmul contracts
                # only finite values. rec/recb are filled per pair so the bc
                # matmul's input is ready long before the deferred normalize
                # drains.
                dq = npool.tile([64, 1024], F32, tag="den")
                nc.gpsimd.memset(dq, 1.0)
                rec = npool.tile([64, 1024], F32, tag="rec")
                recb = npool.tile([64, 1024], BF16, tag="recb")
                nfull = 4 * qt
                for p in range(2):
                    # --- scores: K=64 row-tiled, heads 2p (array rows 0-63)
                    # and 2p+1 (rows 64-127) run CONCURRENTLY, writing the
                    # two PSUM banks of one [P,1024] tile (head A cols 0-512,
                    # head B cols 512-1024) so a single wide exp covers the
                    # pair and the 2-deep sp rotation keeps pipelining.
                    # Diagonal first (maximum slack for its exp+mask chains):
                    # d1=[A-t0|B-t0], d2=[A-t1|B-t1], d3=[A-t2,A-t3|B-t2,B-t3]
                    def pair_mm(sp_, t, o_, w_, qoff):
                        for hh in range(2):
                            r0 = 64 * hh
                            nc.tensor.matmul(
                                sp_[:, ds(512 * hh + o_, w_)],
                                kdup[r0 : r0 + 64, ts(nfull + t, P)],
                                q4[r0 : r0 + 64, p, ds(512 * qt + qoff, w_)],
                                start=True,
                                stop=True,
                            )
                    dpr = []
                    for pieces, ecols in (
                        ([(0, 0, 512, 0)], 1024),
                        ([(1, 0, 384, P)], 896),
                        ([(2, 0, 256, 2 * P), (3, 256, 128, 3 * P)], 896),
                    ):
                        sp_ = pp_sp.tile([P, 1024], F32, tag="sp", name="spd")
                        for t, o_, w_, qoff in pieces:
                            pair_mm(sp_, t, o_, w_, qoff)
                        pr_ = ppool.tile([P, 1024], BF16, tag="pr")
                        nc.scalar.activation(
                            pr_[:, 0:ecols], sp_[:, 0:ecols], FA.Exp, scale=SCALE
                        )
                        for _, o_, w_, _q in pieces:
                            for hh in range(2):
                                # causal mask on the diagonal 128-col chunk:
                                # keep where (q - k) >= 0, else 0. Runs on the
                                # near-idle gpsimd engine instead of DVE.
                                m0 = 512 * hh + o_
                                nc.gpsimd.affine_select(
                                    out=pr_[:, ds(m0, P)],
                                    in_=pr_[:, ds(m0, P)],
                                    pattern=[[1, P]],
                                    compare_op=ALU.is_ge,
                                    fill=0.0,
                                    base=0,
                                    channel_multiplier=-1,
                                )
                        dpr.append(pr_)
                        yield
                    # off-diagonal k-tiles: one [P,1024] tile per k-tile,
                    # both heads concurrent, one exp per k-tile
                    prs = []
                    for kt in range(nfull):
                        sp_ = pp_sp.tile([P, 1024], F32, tag="sp", name="spo")
                        for hh in range(2):
                            r0 = 64 * hh
                            nc.tensor.matmul(
                                sp_[:, ts(hh, 512)],
                                kdup[r0 : r0 + 64, ts(kt, P)],
                                q4[r0 : r0 + 64, p, ts(qt, 512)],
                                start=True,
                                stop=True,
                            )
                        pr_ = ppool.tile([P, 1024], BF16, tag="pr")
                        nc.scalar.activation(pr_, sp_, FA.Exp, scale=SCALE)
                        prs.append(pr_)
                        if kt % 4 == 3:
                            yield
                    # --- attnV per head (full-K mode): off-diag then diag
                    for hh in range(2):
                        acc = pp_acc.tile([D + 1, 512], F32, tag="acc")
                        first = True
                        for kt in range(nfull):
                            nc.tensor.matmul(
                                acc,
                                vsb[:, kt, :],
                                prs[kt][:, ts(hh, 512)],
                                start=first,
                                stop=False,
                            )
                            first = False
                            if kt % 4 == 3:
                                yield
                        dsrc = [
                            (dpr[0], 0, 0, 512),
                            (dpr[1], 0, P, 384),
                            (dpr[2], 0, 2 * P, 256),
                            (dpr[2], 256, 3 * P, 128),
                        ]
                        for t, (pt, so, off, w_) in enumerate(dsrc):
                            nc.tensor.matmul(
                                acc[:, ds(off, w_)],
                                vsb[:, nfull + t, :],
                                pt[:, ds(512 * hh + so, w_)],
                                start=first,
                                stop=(t == 3),
                            )
                            first = False
                        yield
                        # evacuate unnormalized attn + denominator row (head
                        # 2p+hh's denominators land at partition 32*hh).
                        # Denominator row first: it gates the deferred
                        # normalize chain.
                        dst = attnsb[64 * hh : 64 * hh + 64, p, ts(qt, 512)]
                        nc.vector.tensor_copy(
                            dq[32 * hh : 32 * hh + 1, ds(512 * p, 512)],
                            acc[D : D + 1, :],
                        )
                        nc.vector.tensor_copy(dst, acc[0:D, :])
                        if hh == 1:
                            # both rows of this pair's denominators are in:
                            # fold them to bf16 reciprocals now, off the
                            # critical path
                            half = ds(512 * p, 512)
                            nc.vector.reciprocal_approx_fast(rec[:, half], dq[:, half])
                            nc.vector.tensor_copy(recb[:, half], rec[:, half])
                        yield
                state[("pending_norm", b)] = gen_norm(b, qt, recb)

        def gen_norm(b, qt, recb):
            """deferred softmax normalization for q-block qt: one K=64
            selector matmul (2 PSUM-bank halves) broadcasts both feature
            blocks' 1/denom rows across partitions, two multiplies apply
            them. Drained several steps into the NEXT q-block so the bc
            matmul queues behind fresh PE work while the reciprocal chain
            (already emitted per-half) drains."""
            attnsb = state[b][4]
            bc = pp_sp.tile([P, 1024], F32, tag="sp")
            for kk in range(2):
                nc.tensor.matmul(
                    bc[:, ts(kk, 512)], sel2, recb[:, ts(kk, 512)],
                    start=True, stop=True,
                )
            state[("normed", b)][0] += 1
            yield
            for kk in range(2):
                dst = attnsb[:, kk, ts(qt, 512)]
                nc.vector.tensor_tensor(dst, dst, bc[:, ts(kk, 512)], ALU.mult)
            yield

        def gen_outproj(b, use_act, ns=None):
            """partial output projection for batch b (PE-heavy).

            n-outer so the last q-block's softmax-normalize latency is hidden
            behind the first 3 n-blocks' matmuls. use_act alternates the PSUM
            evacuation onto ScalarE only when no attention phase is keeping
            ScalarE saturated with exps.

            Gated on the normalize counter: the normalize path now contains a
            PE matmul (the 1/denom broadcast), so emitting an outproj matmul
            that waits on q-block n's normalize BEFORE that broadcast matmul
            is emitted would deadlock the in-order PE queue.
            """
            attnsb = state[b][4]
            for n in ns if ns is not None else range(NT):
                while state[("normed", b)][0] <= n:
                    yield
                osb = None
                for m in range(KT):
                    po = pp_mm.tile([P, 512], F32, tag="mm")
                    for kk in range(2):
                        nc.tensor.matmul(
                            po,
                            wo_sb[:, kk, ts(m, P)],
                            attnsb[:, kk, ts(n, 512)],
                            start=(kk == 0),
                            stop=(kk == 1),
                        )
                    if m % 2 == 0:
                        osb = opool.tile([P, 1024], BF16, name="osb")
                    odst = osb[:, ts(m % 2, 512)]
                    # use_act None: never touch ScalarE (it paces the
                    # attention phase this generator fills); True: quiet-tail
                    # block, lean on ScalarE to relieve DVE
                    if use_act is True and m % 3 != 0:
                        nc.scalar.copy(odst, po)
                    elif use_act is False and m % 3 == 2:
                        nc.scalar.copy(odst, po)
                    else:
                        nc.vector.tensor_copy(odst, po)
                    if m % 2 == 1:
                        # one partition-major [128,1024] DMA per two m-tiles;
                        # final blocks alternate queues so the closing
                        # transfers drain in parallel
                        if b == 1 and n >= 2 and (m // 2) % 2 == 1:
                            deng = nc.scalar
                        else:
                            deng = nc.sync
                        deng.dma_start(
                            out[b, n, :, ds(m - 1, 2), :],
                            osb.rearrange("p (c s) -> p c s", c=2),
                        )
                    if m % 4 == 3:
                        yield

        def run_all(gen):
            for _ in gen:
                pass

        def interleave(pairs):
            """pairs: list of [gen, steps_per_round]. Round-robin with ratios
            so the PE-filler generator is spread across the whole phase."""
            pairs = [[g, r] for g, r in pairs]
            while pairs:
                for gr in pairs[:]:
                    try:
                        for _ in range(gr[1]):
                            next(gr[0])
                    except StopIteration:
                        pairs.remove(gr)

        def delayed(gen, k):
            for _ in range(k):
                yield
            yield from gen

        def chain(gens):
            for g in gens:
                yield from g

        def att_batch(b):
            """attention for all q-blocks of batch b, draining each block's
            deferred normalize a few steps into the NEXT block (so the bc
            matmul hides behind fresh scores/attnV work), and the final one
            with interleaver turns between its pieces."""
            for qt in range(NT):
                # causal gating: q-block qt only attends keys from
                # projection blocks <= qt, so attention pipelines INTO the
                # projection phase instead of trailing it
                while state[("projed", b)][0] <= qt:
                    yield
                g = gen_att_qt(b, qt)
                steps = 0
                for _ in g:
                    yield
                    steps += 1
                    if steps == 8 and ("pending_norm", b) in state:
                        for _ in state.pop(("pending_norm", b)):
                            yield
            for _ in range(6):
                yield
            for _ in state.pop(("pending_norm", b)):
                yield

        # Pipeline the two batches so PE-heavy projection work fills the PE
        # bubbles of the ACT(exp)-bound attention phases; out-projections
        # enter a phase early, delayed so their first matmuls trail the
        # q-block normalizes they depend on in the in-order PE stream.
        # op0 is split: a delayed sliver covers the tail of the batch-0
        # attention phase (after proj(1) exhausts), the bulk fills the
        # batch-1 attention phase's PE bubbles.
        op0a = gen_outproj(0, None, ns=[0])
        op0b = gen_outproj(0, None, ns=[1, 2, 3])
        # both ops' bulk runs inside exp-saturated attention phases -> keep
        # their evacuations off the scalar engine entirely; only the final
        # n=3 block (the quiet tail) borrows ScalarE
        op1 = gen_outproj(1, None, ns=[0, 1])
        # n=2 held back as dedicated filler for the batch-1 final-normalize
        # window (its bc matmul otherwise stalls the PE with nothing queued)
        op1c = gen_outproj(1, None, ns=[2])
        op1t = gen_outproj(1, True, ns=[3])
        proj = lambda b: chain([gen_proj_n(b, n) for n in range(NT)])
        # batch-0 attention pipelines into the projection phase (causal
        # gating above); proj(1) trails proj(0) via the shared x-buffer
        # pool rotation
        interleave([
            (proj(0), 2),
            (att_batch(0), 4),
            (delayed(proj(1), 16), 1),
            (delayed(op0a, 26), 1),
        ])
        interleave([(op0b, 1), (att_batch(1), 4), (op1, 1), (delayed(op1c, 21), 1)])
        run_all(op1c)
        run_all(op1t)
    return nc


BF = ml_dtypes.bfloat16


def make_in_maps(x, Wq, Wk, Wv, Wo):
    # [B, S, E] -> [B, NT, E, 512] (token-block-tiled, feature-major)
    x_t = np.ascontiguousarray(
        np.transpose(
            np.asarray(x, np.float32).reshape(B, NT, 512, E), (0, 1, 3, 2)
        )
    ).astype(BF)
    Wq = np.asarray(Wq, np.float32)
    Wk = np.asarray(Wk, np.float32)
    Wv = np.asarray(Wv, np.float32)
    Wo = np.asarray(Wo, np.float32)
    in_maps = []
    for c in range(NCORES):
        wq_sh = np.ascontiguousarray(Wq[:, FPC * c : FPC * (c + 1)]).astype(BF)
        wkv_sh = np.concatenate(
            [Wk[:, D * c : D * (c + 1)], Wv[:, D * c : D * (c + 1)]], axis=1
        ).astype(BF)
        wo_sh = np.ascontiguousarray(Wo[FPC * c : FPC * (c + 1), :]).astype(BF)
        in_maps.append({"x_t": x_t, "wq": wq_sh, "wkv": wkv_sh, "wo": wo_sh})
    return in_maps


_NC_CACHE = {}


def get_nc():
    if "nc" not in _NC_CACHE:
        nc = build_nc()
        nc.compile()
        _NC_CACHE["nc"] = nc
    return _NC_CACHE["nc"]


def kernel(x, Wq, Wk, Wv, Wo, bo, mask=None, **_ignored):
    nc = get_nc()
    in_maps = make_in_maps(x, Wq, Wk, Wv, Wo)
    res = run_bass_kernel_spmd(nc, in_maps, list(range(NCORES)))
    total = np.zeros((B, NT, P, KT, 512), np.float32)
    for c in range(NCORES):
        total += np.asarray(res.results[c]["out"], np.float32)
    # [B, NT, 128, KT, 512] -> [B, S, E]: feature = m*128+p, token = n*512+s
    full = np.transpose(total, (0, 1, 4, 3, 2)).reshape(B, S, E)
    full = full + np.asarray(bo, np.float32)[None, None, :]
    return np.ascontiguousarray(full)

